# revision 12
# baseline (speedup 1.0000x reference)
# Trainium2 Bass kernel for CustomLSTMModel (V=32000, E=256, H=512, O=2, S=512, B=64)
#
# Strategy: data-parallel over batch (8 cores x B_loc=8). Weights replicated and
# SBUF-resident. Embedding rows gathered on-device with a transposed dma_gather
# (feature-major layout). Input projections G_x = W_x @ x_t + b precomputed for
# all timesteps in chunks of 64 steps, interleaved with the recurrence on the PE.
# The serial recurrence runs per-core: per step, gates = W_h @ h in a folded
# [128, (gate, jtile, b)] PSUM layout via 64 [128x128]x[128x8] matmuls (weights
# stationary), then sigmoid/tanh on ScalarE and the c/h updates on VectorE.
# c state is kept in fp32; W_h/W_x/h/x are bf16 (fp32 fallback available).

import numpy as np
import ml_dtypes

S, B, V, E, H, O = 512, 64, 32000, 256, 512, 2
NCORES = 8
BLOC = B // NCORES          # 8 batch elements per core
CHUNK = 64                  # timesteps per G_x precompute chunk
NCHUNK = S // CHUNK
MT = 16                     # gate-row tiles (4 gates x 4 tiles of 128)
KT = H // 128               # 4 contraction tiles over h
KC = E // 128               # 2 contraction tiles over x
NIDX = S * BLOC             # 4096 tokens gathered per core

_CACHE = {}


def _build_bass(prec, n_steps):
    import concourse.bass as bass
    import concourse.bacc as bacc
    import concourse.tile as tile
    import concourse.mybir as mybir
    from contextlib import ExitStack

    AF = mybir.ActivationFunctionType
    fp32 = mybir.dt.float32
    bf16 = mybir.dt.bfloat16
    dt = bf16 if prec == "bf16" else fp32

    nc = bacc.Bacc("TRN2")
    idx_d = nc.declare_dram_parameter("idx", [128, NIDX // 128], mybir.dt.int32, isOutput=False)
    emb_d = nc.declare_dram_parameter("embt", [V, E], fp32, isOutput=False)
    ident_d = nc.declare_dram_parameter("ident", [128, 128], fp32, isOutput=False)
    whT_d = nc.declare_dram_parameter("whT", [128, KT * 2048], dt, isOutput=False)
    wxT_d = nc.declare_dram_parameter("wxT", [128, KC * 2048], dt, isOutput=False)
    bf_d = nc.declare_dram_parameter("bfold", [128, MT], fp32, isOutput=False)
    wy_d = nc.declare_dram_parameter("wyT", [128, KT * O], fp32, isOutput=False)
    by_d = nc.declare_dram_parameter("byT", [1, O], fp32, isOutput=False)
    y_d = nc.declare_dram_parameter("y", [BLOC, O], fp32, isOutput=True)

    with tile.TileContext(nc) as tc, ExitStack() as ctx:
        const = ctx.enter_context(tc.tile_pool(name="const", bufs=1))
        gxp = ctx.enter_context(tc.tile_pool(name="gx", bufs=2))
        hp = ctx.enter_context(tc.tile_pool(name="h", bufs=2))
        cp = ctx.enter_context(tc.tile_pool(name="c", bufs=2))
        wk = ctx.enter_context(tc.tile_pool(name="wk", bufs=2))
        psg = ctx.enter_context(tc.tile_pool(name="psg", bufs=2, space="PSUM"))
        psx = ctx.enter_context(tc.tile_pool(name="psx", bufs=2, space="PSUM"))
        psy = ctx.enter_context(tc.tile_pool(name="psy", bufs=1, space="PSUM"))

        idx_sb = const.tile([128, NIDX // 128], mybir.dt.int32)
        nc.sync.dma_start(idx_sb[:], idx_d[:])
        ident = const.tile([128, 128], fp32)
        nc.sync.dma_start(ident[:], ident_d[:])
        whT = const.tile([128, KT * 2048], dt)
        nc.sync.dma_start(whT[:], whT_d[:])
        wxT = const.tile([128, KC * 2048], dt)
        nc.sync.dma_start(wxT[:], wxT_d[:])
        bfold = const.tile([128, MT], fp32)
        nc.sync.dma_start(bfold[:], bf_d[:])
        wyT = const.tile([128, KT * O], fp32)
        nc.sync.dma_start(wyT[:], wy_d[:])
        byT = const.tile([1, O], fp32)
        nc.sync.dma_start(byT[:], by_d[:])
        ones = const.tile([1, BLOC], fp32)
        nc.gpsimd.memset(ones[:], 1.0)

        # Gather embedding rows (tokens on partitions), then PE-transpose into the
        # feature-major layout xg[p, c, i] = emb[tok_i, c*128+p], i = t*BLOC + b.
        xg = const.tile([128, KC, NIDX], dt)
        gp = ctx.enter_context(tc.tile_pool(name="gp", bufs=3))
        pst = ctx.enter_context(tc.tile_pool(name="pst", bufs=2, space="PSUM"))
        for g in range(NIDX // 128):
            xrows = gp.tile([128, E], fp32, tag="xrows")
            nc.gpsimd.indirect_dma_start(
                out=xrows[:],
                out_offset=None,
                in_=emb_d[:, :],
                in_offset=bass.IndirectOffsetOnAxis(ap=idx_sb[:, g : g + 1], axis=0),
            )
            for kc in range(KC):
                pt = pst.tile([128, 128], fp32, tag="pt")
                nc.tensor.transpose(pt[:], xrows[:, kc * 128 : (kc + 1) * 128], ident[:])
                eng = nc.vector if (g + kc) % 2 == 0 else nc.scalar
                if eng is nc.vector:
                    nc.vector.tensor_copy(xg[:, kc, g * 128 : (g + 1) * 128], pt[:])
                else:
                    nc.scalar.copy(xg[:, kc, g * 128 : (g + 1) * 128], pt[:])

        n_chunks = (n_steps + CHUNK - 1) // CHUNK
        gx_tiles = {}

        def emit_gx(c, m):
            # G_x for chunk c, gate-row tile m: one PSUM [128, 64*8] over 2 k-tiles,
            # then bias-add copy into the chunk buffer at columns t*128 + m*8 + b.
            if m == 0:
                gx_tiles[c] = gxp.tile([128, CHUNK * 128], fp32, tag="gx", name=f"gx{c}")
            ps = psx.tile([128, CHUNK * BLOC], fp32, tag="psx")
            for kc in range(KC):
                nc.tensor.matmul(
                    ps[:],
                    wxT[:, kc * 2048 + m * 128 : kc * 2048 + (m + 1) * 128],
                    xg[:, kc, c * CHUNK * BLOC : (c + 1) * CHUNK * BLOC],
                    start=(kc == 0),
                    stop=(kc == KC - 1),
                )
            dst = gx_tiles[c][:].rearrange("p (t mm) -> p t mm", t=CHUNK)[:, :, m * BLOC : (m + 1) * BLOC]
            src = ps[:].rearrange("p (t b) -> p t b", t=CHUNK)
            nc.scalar.add(dst, src, bfold[:, m : m + 1])

        for m in range(MT):
            emit_gx(0, m)

        h_cur = None
        c_cur = None
        for t in range(n_steps):
            c = t // CHUNK
            tl = t % CHUNK
            psg_t = None
            if t > 0:
                psg_t = psg.tile([128, MT * BLOC], fp32, tag="psg")
                for m in range(MT):
                    for k in range(KT):
                        nc.tensor.matmul(
                            psg_t[:, m * BLOC : (m + 1) * BLOC],
                            whT[:, k * 2048 + m * 128 : k * 2048 + (m + 1) * 128],
                            h_cur[:, k * BLOC : (k + 1) * BLOC],
                            start=(k == 0),
                            stop=(k == KT - 1),
                        )
            # interleave next chunk's input-projection matmuls into the PE stream
            if c + 1 < n_chunks and tl < 2 * MT and tl % 2 == 0:
                emit_gx(c + 1, tl // 2)

            gates = wk.tile([128, 128], fp32, tag="gates")
            if t == 0:
                nc.vector.tensor_copy(gates[:], gx_tiles[0][:, 0:128])
            else:
                nc.vector.tensor_add(gates[:], psg_t[:], gx_tiles[c][:, tl * 128 : (tl + 1) * 128])

            gact = wk.tile([128, 128], fp32, tag="gact")
            nc.scalar.activation(gact[:, 0:96], gates[:, 0:96], AF.Sigmoid)
            nc.scalar.activation(gact[:, 96:128], gates[:, 96:128], AF.Tanh)

            c_new = cp.tile([128, KT * BLOC], fp32, tag="c")
            if t == 0:
                nc.vector.tensor_mul(c_new[:], gact[:, 32:64], gact[:, 96:128])
            else:
                t1 = wk.tile([128, KT * BLOC], fp32, tag="t1")
                nc.vector.tensor_mul(t1[:], c_cur[:], gact[:, 0:32])
                t2 = wk.tile([128, KT * BLOC], fp32, tag="t2")
                nc.vector.tensor_mul(t2[:], gact[:, 32:64], gact[:, 96:128])
                nc.vector.tensor_add(c_new[:], t1[:], t2[:])
            tcv = wk.tile([128, KT * BLOC], fp32, tag="tc")
            nc.scalar.activation(tcv[:], c_new[:], AF.Tanh)
            h_new = hp.tile([128, KT * BLOC], dt, tag="h")
            nc.vector.tensor_mul(h_new[:], gact[:, 64:96], tcv[:])
            h_cur, c_cur = h_new, c_new

        # y = h^T @ Wy^T + by  (fp32)
        if prec == "bf16":
            h32 = wk.tile([128, KT * BLOC], fp32, tag="h32")
            nc.vector.tensor_copy(h32[:], h_cur[:])
        else:
            h32 = h_cur
        psy_t = psy.tile([BLOC, O], fp32)
        for j in range(KT):
            nc.tensor.matmul(
                psy_t[:],
                h32[:, j * BLOC : (j + 1) * BLOC],
                wyT[:, j * O : (j + 1) * O],
                start=(j == 0),
                stop=False,
            )
        nc.tensor.matmul(psy_t[:], ones[:], byT[:], start=False, stop=True)
        y_sb = wk.tile([BLOC, O], fp32, tag="ysb")
        nc.vector.tensor_copy(y_sb[:], psy_t[:])
        nc.sync.dma_start(y_d[:], y_sb[:])

    nc.compile()
    return nc


def _prep_inputs(texts, emb, Wf, bf, Wi, bi, Wo, bo, Wc, bc, Wy, by, prec):
    bf16 = ml_dtypes.bfloat16
    wdt = bf16 if prec == "bf16" else np.float32

    texts = np.asarray(texts)
    emb = np.asarray(emb, dtype=np.float32)

    Wall = np.concatenate(
        [np.asarray(Wf), np.asarray(Wi), np.asarray(Wo), np.asarray(Wc)], axis=0
    ).astype(np.float32)  # [2048, 768]
    Wh = Wall[:, :H]      # [2048, 512] multiplies h
    Wx = Wall[:, H:]      # [2048, 256] multiplies x
    ball = np.concatenate(
        [np.asarray(bf), np.asarray(bi), np.asarray(bo), np.asarray(bc)], axis=0
    ).astype(np.float32).reshape(-1)  # [2048]

    # whT[p, k*2048 + m*128 + q] = Wh[m*128+q, k*128+p]
    whT = np.ascontiguousarray(Wh.reshape(MT, 128, KT, 128).transpose(3, 2, 0, 1).reshape(128, KT * 2048)).astype(wdt)
    # wxT[p, kc*2048 + m*128 + q] = Wx[m*128+q, kc*128+p]
    wxT = np.ascontiguousarray(Wx.reshape(MT, 128, KC, 128).transpose(3, 2, 0, 1).reshape(128, KC * 2048)).astype(wdt)
    bfold = np.ascontiguousarray(ball.reshape(MT, 128).T).astype(np.float32)  # [128, 16]
    Wy = np.asarray(Wy, dtype=np.float32)  # [2, 512]
    wyT = np.ascontiguousarray(Wy.reshape(O, KT, 128).transpose(2, 1, 0).reshape(128, KT * O)).astype(np.float32)
    byT = np.asarray(by, dtype=np.float32).reshape(1, O)

    ident = np.eye(128, dtype=np.float32)

    per_core = []
    for ci in range(NCORES):
        sl = texts[:, ci * BLOC : (ci + 1) * BLOC]  # [S, BLOC]
        flat = np.ascontiguousarray(sl).reshape(-1).astype(np.int64)  # i = t*BLOC + b
        idx = np.ascontiguousarray(flat.astype(np.int32).reshape(NIDX // 128, 128).T)
        per_core.append(
            {
                "idx": idx,
                "embt": emb,
                "ident": ident,
                "whT": whT,
                "wxT": wxT,
                "bfold": bfold,
                "wyT": wyT,
                "byT": byT,
            }
        )
    return per_core


def _get_nc(prec="bf16", n_steps=S):
    key = (prec, n_steps)
    if key not in _CACHE:
        _CACHE[key] = _build_bass(prec, n_steps)
    return _CACHE[key]


def kernel(**inputs):
    prec = inputs.pop("_prec", "bf16")
    from concourse.bass_utils import run_bass_kernel_spmd

    nc = _get_nc(prec)
    in_maps = _prep_inputs(
        inputs["texts"], inputs["emb"],
        inputs["Wf"], inputs["bf"], inputs["Wi"], inputs["bi"],
        inputs["Wo"], inputs["bo"], inputs["Wc"], inputs["bc"],
        inputs["Wy"], inputs["by"], prec,
    )
    res = run_bass_kernel_spmd(nc, in_maps, list(range(NCORES)))
    y = np.concatenate([np.asarray(res.results[i]["y"]) for i in range(NCORES)], axis=0)
    return y.astype(np.float32)
